# revision 5
# baseline (speedup 1.0000x reference)
"""Fuzzy-antecedent kernel: out[i, r] = prod_j m_j[i, ri[r, j]] on 8 TRN2 cores.

r = i0*625 + i1*125 + i2*25 + i3*5 + i4 (lexicographic meshgrid over 5 sets
of 5), so each output row is the Kronecker product of the five 5-element
membership rows. Data-parallel over the sample axis: 16384 rows -> 2048 per
core -> 16 partition-tiles of 128. Per tile the product chain is built with
widths 25 -> 125 -> 625 -> 3125 via tensor_scalar multiplies (per-partition
scalar operand), final stage split across the DVE and ACT engines. The
output write (25.6 MB/core) is the roofline.
"""

import numpy as np

import concourse.bass as bass
import concourse.tile as tile
from concourse import bacc, mybir
from concourse.bass_utils import run_bass_kernel_spmd

N = 16384
N_CORES = 8
NPC = N // N_CORES  # 2048 rows per core
NT = NPC // 128  # 16 partition tiles per core
R = 3125
F32 = mybir.dt.float32

# Even-width padding: fp32 tensor_scalar on DVE hits 2x_2P mode only for
# even innermost dims, so padded segments are written one element wide into
# the next segment (ascending order self-heals) and tiles get a pad column.


def build_bass():
    nc = bacc.Bacc()
    # mcat[p, t*25 + j*5 + k] = m_j[t*128 + p, k] (host pre-packed)
    mcat = nc.declare_dram_parameter("mcat", [128, NT * 25], F32, isOutput=False)
    out = nc.declare_dram_parameter("out", [NPC, R], F32, isOutput=True)

    with tile.TileContext(nc) as tc:
        with (
            tc.tile_pool(name="inp", bufs=1) as ipool,
            tc.tile_pool(name="mid", bufs=2) as mpool,
            tc.tile_pool(name="obuf", bufs=4) as opool,
        ):
            mt = ipool.tile([128, NT * 25], F32)
            nc.sync.dma_start(out=mt[:], in_=mcat[:])

            def bc_outer(ap, reps):
                # [p, w] -> [p, w, reps] with stride-0 inner (each elem repeated)
                return ap.broadcast_to([128, ap.shape[1], reps])

            def bc_tile(ap, reps):
                # [p, w] -> [p, reps, w] with stride-0 outer (whole vec tiled)
                return bass.AP(
                    tensor=ap.tensor,
                    offset=ap.offset,
                    ap=[ap.ap[0], [0, reps], list(ap.ap[1])],
                )

            for t in range(NT):
                b = t * 25  # m0 at b, m1 at b+5, m2 at b+10, m3 at b+15, m4 at b+20
                s2 = mpool.tile([128, 25], F32, tag="s2")
                s3 = mpool.tile([128, 125], F32, tag="s3")
                s4 = mpool.tile([128, 626], F32, tag="s4")
                ot = opool.tile([128, R + 1], F32, tag="ot")

                # Single broadcast TT per stage: s_{k+1}[a*w+c] = m[a]*s_k[c]
                nc.vector.tensor_tensor(
                    out=s2[:].rearrange("p (a c) -> p a c", a=5),
                    in0=bc_outer(mt[:, b + 15 : b + 20], 5),
                    in1=bc_tile(mt[:, b + 20 : b + 25], 5),
                    op=mybir.AluOpType.mult,
                )
                nc.vector.tensor_tensor(
                    out=s3[:].rearrange("p (a c) -> p a c", a=5),
                    in0=bc_outer(mt[:, b + 10 : b + 15], 25),
                    in1=bc_tile(s2[:], 5),
                    op=mybir.AluOpType.mult,
                )
                nc.vector.tensor_tensor(
                    out=s4[:, 0:625].rearrange("p (a c) -> p a c", a=5),
                    in0=bc_outer(mt[:, b + 5 : b + 10], 125),
                    in1=bc_tile(s3[:], 5),
                    op=mybir.AluOpType.mult,
                )
                # ot = m0 (x) s4: segments 0-2 on ACT (exact width),
                # segments 3-4 on DVE (padded to even FD for 2x mode; 3
                # stomps 4's first col, 4 stomps the pad col — both
                # DVE-internal, in order).
                for i in range(3):
                    nc.scalar.activation(
                        ot[:, i * 625 : (i + 1) * 625],
                        s4[:, 0:625],
                        mybir.ActivationFunctionType.Copy,
                        scale=mt[:, b + i : b + i + 1],
                    )
                for i in range(3, 5):
                    nc.vector.tensor_scalar_mul(
                        ot[:, i * 625 : i * 625 + 626],
                        s4[:, 0:626],
                        mt[:, b + i : b + i + 1],
                    )
                nc.sync.dma_start(out=out[t * 128 : (t + 1) * 128, :], in_=ot[:, 0:R])
    nc.compile()
    return nc


def _pack_inputs(inputs):
    m = [np.asarray(inputs[f"m{j}"], dtype=np.float32) for j in range(5)]
    cat = np.concatenate(m, axis=1)  # (N, 25), col j*5+k = m_j[:, k]
    cat = cat.reshape(N_CORES, NT, 128, 25)
    packed = np.ascontiguousarray(cat.transpose(0, 2, 1, 3).reshape(N_CORES, 128, NT * 25))
    return [{"mcat": packed[c]} for c in range(N_CORES)]


_CACHED_NC = None


def kernel(**inputs) -> np.ndarray:
    global _CACHED_NC
    in_maps = _pack_inputs(inputs)
    if _CACHED_NC is None:
        _CACHED_NC = build_bass()
    res = run_bass_kernel_spmd(_CACHED_NC, in_maps, core_ids=list(range(N_CORES)))
    return np.concatenate([res.results[c]["out"] for c in range(N_CORES)], axis=0)


# revision 13
# speedup vs baseline: 1.2009x; 1.2009x over previous
"""Fuzzy-antecedent kernel: out[i, r] = prod_j m_j[i, ri[r, j]] on 8 TRN2 cores.

r = i0*625 + i1*125 + i2*25 + i3*5 + i4 (lexicographic meshgrid over 5 sets
of 5), so each output row is the Kronecker product of the five 5-element
membership rows. Data-parallel over the sample axis: 16384 rows -> 2048 per
core -> 16 partition-tiles of 128. Per tile the product chain is built with
widths 25 -> 125 -> 625 via single broadcast tensor_tensor multiplies on
DVE, and the final 625 -> 3125 stage is split between the ACT engine
(segments 0-2, activation-Copy with per-partition scale) and DVE (segments
3-4, tensor_scalar at 2x mode via even-width overlapped writes). The output
write (25.6 MB/core) is the HBM roofline; raw bacc (no TileContext) avoids
the Tile end-barrier and lets the input load overlap first compute.
"""

import numpy as np

import concourse.bass as bass
from concourse import bacc, mybir

N = 16384
N_CORES = 8
NPC = N // N_CORES  # 2048 rows per core
NT = NPC // 128  # 16 partition tiles per core
R = 3125
F32 = mybir.dt.float32

B_OT = 6  # output-tile ring depth
B_S4 = 3  # s4 ring depth
# input DMA chunks (in tiles): tile 0 alone so compute starts early
IN_CHUNKS = [(0, 1), (1, 4), (4, NT)]


def _bc_outer(ap, reps):
    # [p, w] -> [p, w, reps] stride-0 inner (each element repeated)
    return ap.broadcast_to([128, ap.shape[1], reps])


def _bc_tile(ap, reps):
    # [p, w] -> [p, reps, w] stride-0 outer (whole vector tiled)
    return bass.AP(
        tensor=ap.tensor,
        offset=ap.offset,
        ap=[ap.ap[0], [0, reps], list(ap.ap[1])],
    )


def build_bass():
    nc = bacc.Bacc()
    # mcat[p, t*25 + j*5 + k] = m_j[t*128 + p, k] (host pre-packed)
    mcat = nc.declare_dram_parameter("mcat", [128, NT * 25], F32, isOutput=False)
    out = nc.declare_dram_parameter("out", [NPC, R], F32, isOutput=True)

    import contextlib

    with contextlib.ExitStack() as ctx:
        mt = ctx.enter_context(nc.sbuf_tensor([128, NT * 25], F32))
        s2 = ctx.enter_context(nc.sbuf_tensor([128, 25], F32))
        s3 = ctx.enter_context(nc.sbuf_tensor([128, 125], F32))
        s4 = ctx.enter_context(nc.sbuf_tensor([128, B_S4 * 626], F32))
        ot = ctx.enter_context(nc.sbuf_tensor([128, B_OT * (R + 1)], F32))
        sem_in = [ctx.enter_context(nc.semaphore(f"in{c}")) for c in range(len(IN_CHUNKS))]
        sem_dv = ctx.enter_context(nc.semaphore("dv"))
        sem_a = ctx.enter_context(nc.semaphore("a"))
        sem_o = [ctx.enter_context(nc.semaphore(f"o{s}")) for s in range(B_OT)]
        block = ctx.enter_context(nc.Block())

        def tile_chunk(t):
            return next(c for c, (a, b) in enumerate(IN_CHUNKS) if a <= t < b)

        def s4ap(t, lo, hi):
            return s4[:, t % B_S4 * 626 + lo : t % B_S4 * 626 + hi]

        def otap(t, lo, hi):
            return ot[:, t % B_OT * (R + 1) + lo : t % B_OT * (R + 1) + hi]

        @block.sync
        def _(sync):
            for c, (a, b) in enumerate(IN_CHUNKS):
                sync.dma_start(
                    out=mt[:, a * 25 : b * 25], in_=mcat[:, a * 25 : b * 25]
                ).then_inc(sem_in[c], 16)
            for t in range(NT):
                sync.wait_ge(sem_dv, 5 * t + 5)  # DVE ot part done
                sync.wait_ge(sem_a, t + 1)
                sync.dma_start(
                    out=out[t * 128 : (t + 1) * 128, :], in_=otap(t, 0, R)
                ).then_inc(sem_o[t % B_OT], 16)
            for s in range(B_OT):
                uses = len(range(s, NT, B_OT))
                sync.wait_ge(sem_o[s], 16 * uses)

        @block.vector
        def _(vector):
            # DVE in-order dispatch does NOT order a later op's reads/writes
            # against an earlier op's in-flight writes — chain every op on a
            # self-semaphore (what Tile emits).
            dv = [0]

            def chain(ins):
                if dv[0] > 0:
                    ins._wait_ge(sem_dv, dv[0])
                ins.then_inc(sem_dv, 1)
                dv[0] += 1
                return ins

            last_chunk = -1
            for t in range(NT):
                b = t * 25
                c = tile_chunk(t)
                if c > last_chunk:
                    vector.wait_ge(sem_in[c], 16)
                    last_chunk = c
                if t >= B_S4:
                    # s4 slot last read by ACT at tile t-B_S4
                    vector.wait_ge(sem_a, t - B_S4 + 1)
                if t >= B_OT:
                    vector.wait_ge(sem_o[t % B_OT], 16 * (t // B_OT))
                chain(
                    nc.vector.tensor_tensor(
                        out=s2[:].rearrange("p (a c) -> p a c", a=5),
                        in0=_bc_outer(mt[:, b + 15 : b + 20], 5),
                        in1=_bc_tile(mt[:, b + 20 : b + 25], 5),
                        op=mybir.AluOpType.mult,
                    )
                )
                chain(
                    nc.vector.tensor_tensor(
                        out=s3[:].rearrange("p (a c) -> p a c", a=5),
                        in0=_bc_outer(mt[:, b + 10 : b + 15], 25),
                        in1=_bc_tile(s2[:], 5),
                        op=mybir.AluOpType.mult,
                    )
                )
                chain(
                    nc.vector.tensor_tensor(
                        out=s4ap(t, 0, 625).rearrange("p (a c) -> p a c", a=5),
                        in0=_bc_outer(mt[:, b + 5 : b + 10], 125),
                        in1=_bc_tile(s3[:], 5),
                        op=mybir.AluOpType.mult,
                    )
                )  # sem_dv -> 5t+3: s4 ready
                # final-stage segments 3-4 (padded width 626 for 2x mode;
                # seg 3 stomps seg 4's first col, seg 4 stomps the pad col)
                for i in range(3, 5):
                    chain(
                        nc.vector.tensor_scalar_mul(
                            otap(t, i * 625, i * 625 + 626),
                            s4ap(t, 0, 626),
                            mt[:, b + i : b + i + 1],
                        )
                    )  # sem_dv -> 5t+5: ot DVE part done

        @block.scalar
        def _(scalar):
            for t in range(NT):
                b = t * 25
                scalar.wait_ge(sem_dv, 5 * t + 3)  # s4 ready
                if t >= B_OT:
                    scalar.wait_ge(sem_o[t % B_OT], 16 * (t // B_OT))
                for i in range(3):
                    ins = nc.scalar.activation(
                        otap(t, i * 625, (i + 1) * 625),
                        s4ap(t, 0, 625),
                        mybir.ActivationFunctionType.Copy,
                        scale=mt[:, b + i : b + i + 1],
                    )
                ins.then_inc(sem_a, 1)  # -> t+1

    nc.compile()
    return nc


def _pack_inputs(inputs):
    m = [np.asarray(inputs[f"m{j}"], dtype=np.float32) for j in range(5)]
    cat = np.concatenate(m, axis=1)  # (N, 25), col j*5+k = m_j[:, k]
    cat = cat.reshape(N_CORES, NT, 128, 25)
    packed = np.ascontiguousarray(cat.transpose(0, 2, 1, 3).reshape(N_CORES, 128, NT * 25))
    return [{"mcat": packed[c]} for c in range(N_CORES)]


_CACHED_NC = None


def kernel(**inputs) -> np.ndarray:
    global _CACHED_NC
    from concourse.bass_utils import run_bass_kernel_spmd

    in_maps = _pack_inputs(inputs)
    if _CACHED_NC is None:
        _CACHED_NC = build_bass()
    res = run_bass_kernel_spmd(_CACHED_NC, in_maps, core_ids=list(range(N_CORES)))
    return np.concatenate([res.results[c]["out"] for c in range(N_CORES)], axis=0)
